# revision 2
# baseline (speedup 1.0000x reference)
"""Cosformer (linear attention) Trainium2 Bass kernel.

Problem: B=4, H=16, S=4096, D=64 fp32.
  q_cs = [relu(q/8)*cos | relu(q/8)*sin]   (cos/sin of (pi/2)*(s+1)/S)
  k_cs = [relu(k)*cos   | relu(k)*sin]
  kv   = k_cs^T @ v        [2D, D]
  ksum = sum_s k_cs        [2D]
  out  = (q_cs @ kv) / max(q_cs @ ksum, eps)

Sharding: batch*heads = 64 pairs -> 8 pairs per NeuronCore, 8 cores, no
cross-core communication.

Per-core kernel (per pair), all matmul operands bf16, fp32 PSUM accum:
  s is grouped as s = 32*p + n  (p = partition, n = group 0..31), so all
  HBM DMAs are contiguous 8KB-per-partition transfers.
  - K features: k_cs[s, :] built with one relu + two [128,2048] bf16
    tensor_tensor multiplies against precomputed cos/sin tables.
  - Q features (transposed layout, q_csT[2D, s]) built on the PE: per
    group, matmul(lhsT=q_n [128s,64], rhs=diag_n [128s,256]) where diag_n
    holds cos/8 / sin/8 on the diagonal; relu folds into the PSUM->SBUF
    copy on the scalar engine (cos,sin > 0).
  - kv_aug[2D, 65] = sum_n k_cs_n^T @ [v_n | 1] accumulated in PSUM.
  - out_n[s,65] = q_csT_n^T @ kv_aug; col 64 is the norm denominator.
    Final scale by 1/max(denom,eps) via per-bank strided DVE ops.
"""

import numpy as np
import ml_dtypes

B, H, S, D = 4, 16, 4096, 64
NCORES = 8
PAIRS = (B * H) // NCORES  # 8
P = 128
NG = S // P  # 32 groups; s = 32*p + n
D2 = 2 * D  # 128
EPS = 1e-6
GPB = 4  # groups per PSUM bank in q/out stages
NBANK = NG // GPB  # 8

bf16 = ml_dtypes.bfloat16

_cache = {}


def _consts():
    if "consts" in _cache:
        return _cache["consts"]
    ang = (np.pi / 2) * np.arange(1, S + 1, dtype=np.float64) / S
    cosv, sinv = np.cos(ang), np.sin(ang)
    sidx = NG * np.arange(P)[:, None] + np.arange(NG)[None, :]  # s = 32p + n -> [P, NG]
    cpn, spn = cosv[sidx], sinv[sidx]
    cos_tbl = np.broadcast_to(cpn[:, :, None], (P, NG, D)).reshape(P, NG * D)
    sin_tbl = np.broadcast_to(spn[:, :, None], (P, NG, D)).reshape(P, NG * D)
    diag = np.zeros((P, NG, 2 * P), np.float64)
    ii = np.arange(P)
    scale = 1.0 / np.sqrt(D)
    diag[ii, :, ii] = cpn * scale
    diag[ii, :, ii + P] = spn * scale
    out = (
        np.ascontiguousarray(cos_tbl.astype(bf16)),
        np.ascontiguousarray(sin_tbl.astype(bf16)),
        np.ascontiguousarray(diag.reshape(P, NG * 2 * P).astype(bf16)),
    )
    _cache["consts"] = out
    return out


def build_nc(pairs=PAIRS, num_devices=NCORES):
    from contextlib import ExitStack

    import concourse.bacc as bacc
    import concourse.tile as tile
    import concourse.mybir as mybir

    dt = mybir.dt
    A = mybir.AluOpType
    AF = mybir.ActivationFunctionType

    nc = bacc.Bacc(
        "TRN2", target_bir_lowering=False, debug=False, num_devices=num_devices
    )
    qin = nc.dram_tensor("q", [pairs, S, D], dt.float32, kind="ExternalInput").ap()
    kin = nc.dram_tensor("k", [pairs, S, D], dt.float32, kind="ExternalInput").ap()
    vin = nc.dram_tensor("v", [pairs, S, D], dt.float32, kind="ExternalInput").ap()
    ctd = nc.dram_tensor("cost", [P, NG * D], dt.bfloat16, kind="ExternalInput").ap()
    std = nc.dram_tensor("sint", [P, NG * D], dt.bfloat16, kind="ExternalInput").ap()
    dgd = nc.dram_tensor(
        "diag", [P, NG * 2 * P], dt.bfloat16, kind="ExternalInput"
    ).ap()
    odr = nc.dram_tensor("out", [pairs, S, D], dt.float32, kind="ExternalOutput").ap()

    with tile.TileContext(nc) as tc, ExitStack() as ctx:
        cpool = ctx.enter_context(tc.tile_pool(name="consts", bufs=1))
        inpool = ctx.enter_context(tc.tile_pool(name="inp", bufs=2))
        fpool = ctx.enter_context(tc.tile_pool(name="feat", bufs=2))
        opool = ctx.enter_context(tc.tile_pool(name="outp", bufs=2))
        spool = ctx.enter_context(tc.tile_pool(name="small", bufs=8))
        ppq = ctx.enter_context(tc.tile_pool(name="ppq", bufs=2, space="PSUM"))
        ppkv = ctx.enter_context(tc.tile_pool(name="ppkv", bufs=2, space="PSUM"))
        ppo = ctx.enter_context(tc.tile_pool(name="ppo", bufs=3, space="PSUM"))

        ct = cpool.tile([P, NG * D], dt.bfloat16, tag="ct")
        nc.sync.dma_start(ct[:], ctd)
        st = cpool.tile([P, NG * D], dt.bfloat16, tag="st")
        nc.sync.dma_start(st[:], std)
        dg = cpool.tile([P, NG * 2 * P], dt.bfloat16, tag="dg")
        nc.sync.dma_start(dg[:], dgd)
        ct3 = ct[:].rearrange("p (n d) -> p n d", d=D)
        st3 = st[:].rearrange("p (n d) -> p n d", d=D)
        dg3 = dg[:].rearrange("p (n j) -> p n j", j=2 * P)

        for i in range(pairs):
            qb = inpool.tile([P, NG * D], dt.bfloat16, tag="qb")
            nc.gpsimd.dma_start(qb[:], qin[i].rearrange("(p n) d -> p (n d)", p=P))
            kb = inpool.tile([P, NG * D], dt.bfloat16, tag="kb")
            nc.gpsimd.dma_start(kb[:], kin[i].rearrange("(p n) d -> p (n d)", p=P))
            vb = inpool.tile([P, NG * D], dt.bfloat16, tag="vb")
            nc.gpsimd.dma_start(vb[:], vin[i].rearrange("(p n) d -> p (n d)", p=P))

            # ---- K features (natural layout, bf16) ----
            kr = fpool.tile([P, NG * D], dt.bfloat16, tag="kr")
            nc.vector.tensor_scalar(kr[:], kb[:], 0.0, None, A.max)
            kr3 = kr[:].rearrange("p (n d) -> p n d", d=D)
            kcs = fpool.tile([P, NG * D2], dt.bfloat16, tag="kcs")
            kcs3 = kcs[:].rearrange("p (n j) -> p n j", j=D2)
            nc.vector.tensor_tensor(kcs3[:, :, 0:D], kr3, ct3, A.mult)
            nc.vector.tensor_tensor(kcs3[:, :, D:D2], kr3, st3, A.mult)

            # ---- V blocks with appended ones column ----
            VB = fpool.tile([P, NG * 66], dt.bfloat16, tag="VB")
            VB3 = VB[:].rearrange("p (n j) -> p n j", j=66)
            nc.vector.memset(VB3[:, :, D : D + 1], 1.0)
            vb3 = vb[:].rearrange("p (n d) -> p n d", d=D)
            nc.vector.tensor_copy(VB3[:, :, 0:D], vb3)

            # ---- Q^T features via diagonal matmuls, relu on PSUM->SBUF copy ----
            qcsT = fpool.tile([P, NG * P], dt.bfloat16, tag="qcsT")
            qb3 = qb[:].rearrange("p (n d) -> p n d", d=D)
            for bk in range(NBANK):
                psq = ppq.tile([P, GPB * P], dt.float32, tag="psq")
                for g in range(GPB):
                    n = GPB * bk + g
                    nc.tensor.matmul(
                        psq[0:D, g * P : (g + 1) * P],
                        qb3[:, n, :],
                        dg3[:, n, 0:P],
                        start=True,
                        stop=True,
                    )
                    nc.tensor.matmul(
                        psq[D:D2, g * P : (g + 1) * P],
                        qb3[:, n, :],
                        dg3[:, n, P : 2 * P],
                        start=True,
                        stop=True,
                    )
                nc.scalar.activation(
                    qcsT[:, bk * GPB * P : (bk + 1) * GPB * P], psq[:], AF.Relu
                )

            # ---- kv_aug accumulation over groups ----
            pskv = ppkv.tile([P, 65], dt.float32, tag="pskv")
            for n in range(NG):
                nc.tensor.matmul(
                    pskv[:],
                    kcs3[:, n, :],
                    VB3[:, n, 0:65],
                    start=(n == 0),
                    stop=(n == NG - 1),
                )
            kvb = spool.tile([P, 65], dt.bfloat16, tag="kvb")
            nc.vector.tensor_copy(kvb[:], pskv[:])

            # ---- out matmuls + normalization ----
            osb = opool.tile([P, NG * D], dt.float32, tag="osb")
            osb3 = osb[:].rearrange("p (n d) -> p n d", d=D)
            qcsT3 = qcsT[:].rearrange("p (n j) -> p n j", j=P)
            for bk in range(NBANK):
                pso = ppo.tile([P, GPB * 65], dt.float32, tag="pso")
                pso3 = pso[:].rearrange("p (g j) -> p g j", j=65)
                for g in range(GPB):
                    n = GPB * bk + g
                    nc.tensor.matmul(
                        pso3[:, g, :], qcsT3[:, n, :], kvb[:], start=True, stop=True
                    )
                dn = spool.tile([P, GPB], dt.float32, tag="dn")
                nc.vector.tensor_copy(dn[:].unsqueeze(2), pso3[:, :, D : D + 1])
                rr = spool.tile([P, GPB], dt.float32, tag="rr")
                nc.vector.tensor_scalar(rr[:], dn[:], EPS, None, A.max)
                nc.vector.reciprocal(rr[:], rr[:])
                nc.vector.tensor_tensor(
                    osb3[:, GPB * bk : GPB * (bk + 1), :],
                    pso3[:, :, 0:D],
                    rr[:].unsqueeze(2).broadcast_to((P, GPB, D)),
                    A.mult,
                )
            nc.sync.dma_start(odr[i].rearrange("(p n) d -> p (n d)", p=P), osb[:])

    nc.compile()
    return nc


def _get_runner():
    """Build the compiled program + a stable sharded jit callable once."""
    if "runner" in _cache:
        return _cache["runner"]

    import jax
    import concourse.mybir as mybir
    from concourse import bass2jax
    from jax.experimental.shard_map import shard_map
    from jax.sharding import Mesh, PartitionSpec

    nc = build_nc()
    bass2jax.install_neuronx_cc_hook()

    partition_name = nc.partition_id_tensor.name if nc.partition_id_tensor else None
    in_names, out_names, out_avals, zero_outs = [], [], [], []
    for alloc in nc.m.functions[0].allocations:
        if not isinstance(alloc, mybir.MemoryLocationSet):
            continue
        name = alloc.memorylocations[0].name
        if alloc.kind == "ExternalInput":
            if name != partition_name:
                in_names.append(name)
        elif alloc.kind == "ExternalOutput":
            out_names.append(name)
            shape = tuple(alloc.tensor_shape)
            dtype = mybir.dt.np(alloc.dtype)
            out_avals.append(jax.core.ShapedArray(shape, dtype))
            zero_outs.append(np.zeros(shape, dtype))
    n_params = len(in_names)
    all_names = in_names + out_names
    if partition_name is not None:
        all_names = all_names + [partition_name]

    def _body(*args):
        operands = list(args)
        if partition_name is not None:
            operands.append(bass2jax.partition_id_tensor())
        outs = bass2jax._bass_exec_p.bind(
            *operands,
            out_avals=tuple(out_avals),
            in_names=tuple(all_names),
            out_names=tuple(out_names),
            lowering_input_output_aliases=(),
            sim_require_finite=True,
            sim_require_nnan=True,
            nc=nc,
        )
        return tuple(outs)

    devices = jax.devices()[:NCORES]
    mesh = Mesh(np.asarray(devices), ("core",))
    fn = jax.jit(
        shard_map(
            _body,
            mesh=mesh,
            in_specs=(PartitionSpec("core"),) * (n_params + len(out_names)),
            out_specs=(PartitionSpec("core"),) * len(out_names),
            check_rep=False,
        ),
        keep_unused=True,
    )
    runner = (fn, in_names, out_names, out_avals, zero_outs)
    _cache["runner"] = runner
    return runner


def _concat_inputs(query, key, value):
    """Per-core input dict -> concatenated global arrays (axis 0 sharded)."""
    q = np.ascontiguousarray(np.asarray(query, dtype=np.float32).reshape(B * H, S, D))
    k = np.ascontiguousarray(np.asarray(key, dtype=np.float32).reshape(B * H, S, D))
    v = np.ascontiguousarray(np.asarray(value, dtype=np.float32).reshape(B * H, S, D))
    cos_tbl, sin_tbl, diag = _consts()
    per_name = {
        "q": q,  # [64, S, D] -> shards [8, S, D]
        "k": k,
        "v": v,
        "cost": np.concatenate([cos_tbl] * NCORES, axis=0),
        "sint": np.concatenate([sin_tbl] * NCORES, axis=0),
        "diag": np.concatenate([diag] * NCORES, axis=0),
    }
    return per_name


def kernel(query, key, value):
    fn, in_names, out_names, out_avals, zero_outs = _get_runner()
    per_name = _concat_inputs(query, key, value)
    ins = [per_name[n] for n in in_names]
    zeros = [
        np.zeros((NCORES * z.shape[0], *z.shape[1:]), z.dtype) for z in zero_outs
    ]
    outs = fn(*ins, *zeros)
    out = np.asarray(outs[out_names.index("out")])  # [64, S, D]
    return out.reshape(B, H, S, D).astype(np.float32)


# revision 27
# speedup vs baseline: 392.4001x; 392.4001x over previous
"""Cosformer (linear attention) Trainium2 Bass kernel.

Problem: B=4, H=16, S=4096, D=64 fp32.
  q_cs = [relu(q/8)*cos | relu(q/8)*sin]   (cos/sin of (pi/2)*(s+1)/S)
  k_cs = [relu(k)*cos   | relu(k)*sin]
  kv   = k_cs^T @ v        [2D, D]
  ksum = sum_s k_cs        [2D]
  out  = (q_cs @ kv) / max(q_cs @ ksum, eps)

Sharding: batch*heads = 64 pairs -> 8 pairs per NeuronCore, 8 cores, no
cross-core communication.

Per-core kernel (per pair), all matmul operands bf16, fp32 PSUM accum:
  s is grouped as s = 32*p + n  (p = partition, n = group 0..31), so all
  HBM DMAs are contiguous >=8KB-per-partition transfers; q/k/v are cast to
  bf16 by the SWDGE DMA itself (k alone, then [q|v] stacked, so the k
  feature chain starts earliest).
  - K features: k_cs[s, :] built with one relu (alternating ACT/DVE) + two
    [128,2048] bf16 tensor_tensor multiplies against precomputed cos/sin
    tables (layout-matched to the s = 32p+n grouping).
  - Q features (transposed layout, q_csT[2D, s]) built on the PE: per
    group, matmul(lhsT=q_n [128s,64], rhs=diag_n [128s,256]) where diag_n
    holds cos/8 / sin/8 on the diagonal; relu folds into the PSUM->SBUF
    copy on the scalar engine (cos,sin > 0), batched 8 groups / 2 PSUM
    banks per ACT op.
  - kv[2D, 64] and ksum[2D] accumulate in two PSUM tiles: per group one
    N=64 matmul against v_n plus one N=1 matmul against a ones column,
    sharing the k_cs_n stationary back-to-back.
  - out_n[s,65] = q_csT_n^T @ [kv|ksum] (denominator rides along as col
    64); 7 groups per PSUM bank, then one fused tensor_scalar(max eps) +
    reciprocal + one broadcast tensor_tensor multiply writes the
    normalized fp32 output per bank.
"""

import numpy as np
import ml_dtypes

B, H, S, D = 4, 16, 4096, 64
NCORES = 8
PAIRS = (B * H) // NCORES  # 8
P = 128
NG = S // P  # 32 groups; s = 32*p + n
D2 = 2 * D  # 128
EPS = 1e-6
GPB = 4  # groups per PSUM bank in q/out stages
NBANK = NG // GPB  # 8

bf16 = ml_dtypes.bfloat16

_cache = {}


def _consts():
    if "consts" in _cache:
        return _cache["consts"]
    ang = (np.pi / 2) * np.arange(1, S + 1, dtype=np.float64) / S
    cosv, sinv = np.cos(ang), np.sin(ang)
    sidx = NG * np.arange(P)[:, None] + np.arange(NG)[None, :]  # s = 32p + n -> [P, NG]
    cpn, spn = cosv[sidx], sinv[sidx]
    cos_tbl = np.broadcast_to(cpn[:, :, None], (P, NG, D)).reshape(P, NG * D)
    sin_tbl = np.broadcast_to(spn[:, :, None], (P, NG, D)).reshape(P, NG * D)
    diag = np.zeros((P, NG, 2 * P), np.float64)
    ii = np.arange(P)
    scale = 1.0 / np.sqrt(D)
    diag[ii, :, ii] = cpn * scale
    diag[ii, :, ii + P] = spn * scale
    out = (
        np.ascontiguousarray(cos_tbl.astype(bf16)),
        np.ascontiguousarray(sin_tbl.astype(bf16)),
        np.ascontiguousarray(diag.reshape(P, NG * 2 * P).astype(bf16)),
    )
    _cache["consts"] = out
    return out


def build_nc(pairs=PAIRS, num_devices=NCORES, reps=1):
    from contextlib import ExitStack

    import concourse.bacc as bacc
    import concourse.tile as tile
    import concourse.mybir as mybir

    dt = mybir.dt
    A = mybir.AluOpType
    AF = mybir.ActivationFunctionType

    nc = bacc.Bacc(
        "TRN2", target_bir_lowering=False, debug=False, num_devices=num_devices
    )
    kin = nc.dram_tensor("k", [pairs, S, D], dt.float32, kind="ExternalInput").ap()
    qvin = nc.dram_tensor(
        "qv", [pairs, 2, S, D], dt.float32, kind="ExternalInput"
    ).ap()
    ctd = nc.dram_tensor("cost", [P, NG * D], dt.bfloat16, kind="ExternalInput").ap()
    std = nc.dram_tensor("sint", [P, NG * D], dt.bfloat16, kind="ExternalInput").ap()
    dgd = nc.dram_tensor(
        "diag", [P, NG * 2 * P], dt.bfloat16, kind="ExternalInput"
    ).ap()
    odr = nc.dram_tensor("out", [pairs, S, D], dt.float32, kind="ExternalOutput").ap()

    with tile.TileContext(nc) as tc, ExitStack() as ctx:
        cpool = ctx.enter_context(tc.tile_pool(name="consts", bufs=1))
        inpool = ctx.enter_context(tc.tile_pool(name="inp", bufs=4))
        fpool = ctx.enter_context(tc.tile_pool(name="feat", bufs=3))
        opool = ctx.enter_context(tc.tile_pool(name="outp", bufs=2))
        spool = ctx.enter_context(tc.tile_pool(name="small", bufs=8))
        ppq = ctx.enter_context(tc.tile_pool(name="ppq", bufs=2, space="PSUM"))
        ppkv = ctx.enter_context(tc.tile_pool(name="ppkv", bufs=1, space="PSUM"))
        ppo = ctx.enter_context(tc.tile_pool(name="ppo", bufs=2, space="PSUM"))

        ct = cpool.tile([P, NG * D], dt.bfloat16, tag="ct")
        nc.sync.dma_start(ct[:], ctd)
        st = cpool.tile([P, NG * D], dt.bfloat16, tag="st")
        nc.sync.dma_start(st[:], std)
        dg = cpool.tile([P, NG * 2 * P], dt.bfloat16, tag="dg")
        nc.sync.dma_start(dg[:], dgd)
        ones = cpool.tile([P, 1], dt.bfloat16, tag="ones")
        nc.vector.memset(ones[:], 1.0)
        ct3 = ct[:].rearrange("p (n d) -> p n d", d=D)
        st3 = st[:].rearrange("p (n d) -> p n d", d=D)
        dg3 = dg[:].rearrange("p (n j) -> p n j", j=2 * P)

        for i in [i for _ in range(reps) for i in range(pairs)]:
            kbt = inpool.tile([P, NG * D], dt.bfloat16, tag="kb")
            nc.gpsimd.dma_start(kbt[:], kin[i].rearrange("(p n) d -> p (n d)", p=P))
            kb = kbt[:]
            qv = inpool.tile([P, 2 * NG * D], dt.bfloat16, tag="qv")
            nc.gpsimd.dma_start(
                qv[:].rearrange("p (t x) -> p t x", t=2),
                qvin[i].rearrange("t (p n) d -> p t (n d)", p=P),
            )
            qb = qv[:, 0 : NG * D]
            vb = qv[:, NG * D : 2 * NG * D]

            # ---- K features (natural layout, bf16); relu alternates ACT/DVE ----
            kr = fpool.tile([P, NG * D], dt.bfloat16, tag="kr")
            if i % 2 == 0:
                nc.scalar.activation(kr[:], kb, AF.Relu)
            else:
                nc.vector.tensor_scalar(kr[:], kb, 0.0, None, A.max)
            kr3 = kr[:].rearrange("p (n d) -> p n d", d=D)
            kcs = fpool.tile([P, NG * D2], dt.bfloat16, tag="kcs")
            kcs3 = kcs[:].rearrange("p (n j) -> p n j", j=D2)
            nc.vector.tensor_tensor(kcs3[:, :, 0:D], kr3, ct3, A.mult)
            nc.vector.tensor_tensor(kcs3[:, :, D:D2], kr3, st3, A.mult)

            vb3 = vb.rearrange("p (n d) -> p n d", d=D)

            # ---- Q^T features via diagonal matmuls, relu on PSUM->SBUF copy ----
            # 8 groups per 2-bank PSUM tile -> 4 ACT relu-copies per pair.
            qcsT = fpool.tile([P, NG * P], dt.bfloat16, tag="qcsT")
            qb3 = qb.rearrange("p (n d) -> p n d", d=D)
            GQ = 8
            for bk in range(NG // GQ):
                psq = ppq.tile([P, GQ * P], dt.float32, tag="psq")
                for g in range(GQ):
                    n = GQ * bk + g
                    nc.tensor.matmul(
                        psq[0:D, g * P : (g + 1) * P],
                        qb3[:, n, :],
                        dg3[:, n, 0:P],
                        start=True,
                        stop=True,
                    )
                    nc.tensor.matmul(
                        psq[D:D2, g * P : (g + 1) * P],
                        qb3[:, n, :],
                        dg3[:, n, P : 2 * P],
                        start=True,
                        stop=True,
                    )
                nc.scalar.activation(
                    qcsT[:, bk * GQ * P : (bk + 1) * GQ * P], psq[:], AF.Relu
                )

            # ---- kv_aug accumulation over groups ----
            pskv = ppkv.tile([P, D], dt.float32, tag="pskv")
            psks = ppkv.tile([P, 1], dt.float32, tag="psks")
            for n in range(NG):
                nc.tensor.matmul(
                    pskv[:],
                    kcs3[:, n, :],
                    vb3[:, n, :],
                    start=(n == 0),
                    stop=(n == NG - 1),
                )
                nc.tensor.matmul(
                    psks[:],
                    kcs3[:, n, :],
                    ones[:],
                    start=(n == 0),
                    stop=(n == NG - 1),
                )
            kvb = spool.tile([P, 65], dt.bfloat16, tag="kvb")
            nc.vector.tensor_copy(kvb[:, 0:D], pskv[:])
            nc.vector.tensor_copy(kvb[:, D : D + 1], psks[:])

            # ---- out matmuls (N=65, denom in col 64) + per-bank normalize ----
            qcsT3 = qcsT[:].rearrange("p (n j) -> p n j", j=P)
            osb = opool.tile([P, NG * D], dt.float32, tag="osb")
            osb3 = osb[:].rearrange("p (n d) -> p n d", d=D)
            n0 = 0
            while n0 < NG:
                gpb = min(7, NG - n0)
                pso = ppo.tile([P, 7 * 65], dt.float32, tag="pso")
                pso3 = pso[:].rearrange("p (g j) -> p g j", j=65)
                for g in range(gpb):
                    nc.tensor.matmul(
                        pso3[:, g, :],
                        qcsT3[:, n0 + g, :],
                        kvb[:],
                        start=True,
                        stop=True,
                    )
                rr = spool.tile([P, 7], dt.float32, tag="rr")
                nc.vector.tensor_scalar(
                    rr[:, 0:gpb].unsqueeze(2), pso3[:, 0:gpb, D : D + 1], EPS, None, A.max
                )
                nc.vector.reciprocal(rr[:, 0:gpb], rr[:, 0:gpb])
                nc.vector.tensor_tensor(
                    osb3[:, n0 : n0 + gpb, :],
                    pso3[:, 0:gpb, 0:D],
                    rr[:, 0:gpb].unsqueeze(2).broadcast_to((P, gpb, D)),
                    A.mult,
                )
                n0 += gpb
            nc.sync.dma_start(odr[i].rearrange("(p n) d -> p (n d)", p=P), osb[:])

    nc.compile()
    return nc


def _get_runner():
    """Build the compiled program + a stable sharded jit callable once."""
    if "runner" in _cache:
        return _cache["runner"]

    import jax
    import concourse.mybir as mybir
    from concourse import bass2jax
    from jax.experimental.shard_map import shard_map
    from jax.sharding import Mesh, PartitionSpec

    nc = build_nc()
    bass2jax.install_neuronx_cc_hook()

    partition_name = nc.partition_id_tensor.name if nc.partition_id_tensor else None
    in_names, out_names, out_avals, zero_outs = [], [], [], []
    for alloc in nc.m.functions[0].allocations:
        if not isinstance(alloc, mybir.MemoryLocationSet):
            continue
        name = alloc.memorylocations[0].name
        if alloc.kind == "ExternalInput":
            if name != partition_name:
                in_names.append(name)
        elif alloc.kind == "ExternalOutput":
            out_names.append(name)
            shape = tuple(alloc.tensor_shape)
            dtype = mybir.dt.np(alloc.dtype)
            out_avals.append(jax.core.ShapedArray(shape, dtype))
            zero_outs.append(np.zeros(shape, dtype))
    n_params = len(in_names)
    all_names = in_names + out_names
    if partition_name is not None:
        all_names = all_names + [partition_name]

    def _body(*args):
        operands = list(args)
        if partition_name is not None:
            operands.append(bass2jax.partition_id_tensor())
        outs = bass2jax._bass_exec_p.bind(
            *operands,
            out_avals=tuple(out_avals),
            in_names=tuple(all_names),
            out_names=tuple(out_names),
            lowering_input_output_aliases=(),
            sim_require_finite=True,
            sim_require_nnan=True,
            nc=nc,
        )
        return tuple(outs)

    devices = jax.devices()[:NCORES]
    mesh = Mesh(np.asarray(devices), ("core",))
    fn = jax.jit(
        shard_map(
            _body,
            mesh=mesh,
            in_specs=(PartitionSpec("core"),) * (n_params + len(out_names)),
            out_specs=(PartitionSpec("core"),) * len(out_names),
            check_rep=False,
        ),
        keep_unused=True,
    )
    runner = (fn, in_names, out_names, out_avals, zero_outs)
    _cache["runner"] = runner
    return runner


def _concat_inputs(query, key, value):
    """Per-core input dict -> concatenated global arrays (axis 0 sharded)."""
    q = np.ascontiguousarray(np.asarray(query, dtype=np.float32).reshape(B * H, S, D))
    k = np.ascontiguousarray(np.asarray(key, dtype=np.float32).reshape(B * H, S, D))
    v = np.ascontiguousarray(np.asarray(value, dtype=np.float32).reshape(B * H, S, D))
    cos_tbl, sin_tbl, diag = _consts()
    qv = np.ascontiguousarray(np.stack([q, v], axis=1))  # [64, 2, S, D]
    per_name = {
        "k": k,
        "qv": qv,
        "cost": np.concatenate([cos_tbl] * NCORES, axis=0),
        "sint": np.concatenate([sin_tbl] * NCORES, axis=0),
        "diag": np.concatenate([diag] * NCORES, axis=0),
    }
    return per_name


def kernel(query, key, value):
    fn, in_names, out_names, out_avals, zero_outs = _get_runner()
    per_name = _concat_inputs(query, key, value)
    ins = [per_name[n] for n in in_names]
    zeros = [
        np.zeros((NCORES * z.shape[0], *z.shape[1:]), z.dtype) for z in zero_outs
    ]
    outs = fn(*ins, *zeros)
    out = np.asarray(outs[out_names.index("out")])  # [64, S, D]
    return out.reshape(B, H, S, D).astype(np.float32)
